# revision 2
# baseline (speedup 1.0000x reference)
"""Trainium2 Bass kernel v2 for nn_Block (dense transformer, sigmoid attention).

Sharding: 8 cores = 2 (batch) x 4 (query-chunk of 512 tokens), host-rotated
token axis per core (attention output is key-order invariant).

v2 changes vs v1 (all driven by the TimelineSim cost model, where a matmul
costs out_free_size * PE_CYCLE * cycles_per_row regardless of contract dim,
and fp8e4 DoubleRow runs at 0.5 cycles/row contracting 2x128 slabs):

- fp8e4 DoubleRow for Q/K projections, the score matmuls, and the self
  projection. q/k live in a [32-partition-group, 2-slab] layout so the
  64-wide head contraction maps onto DoubleRow (32x2). Power-of-2 scales
  (wq*2^13, wk*2^10, psum-copy scales 2^-8/2^-5) keep fp8 out of the
  subnormal range; the net 2^-10 is folded into the sigmoid's per-partition
  scale operand along with the key-token rstd. Verified numerically:
  max-rel-err ~1.2e-2 < 2e-2 budget. v/attV/fc/fcproj stay bf16 (fp8 there
  measured over budget).
- attV computed transposed: y^T[q,f] = s^T v with s as the stationary
  operand, out free dim = 64 -> FLOP-optimal 13.7us instead of 27.3us.
  y^T accumulates in 2 PSUM banks per query-half pass (self projection
  opens the accumulation), no flush-adds.
- two query-half passes; pass 1's ACT-bound window hides the first half's
  entire epilogue (proj/LN2/MLP emitted interleaved with pass-1 units).
- gelu computed as x*(1+erf(x/sqrt2)) (0.5 folded into wfcp on the host):
  erf shares the sigmoid activation table, so no table thrash when gelu
  lands between pass-1 sigmoids.
- coulomb multiply stays tensor_tensor (2x_1p DVE mode).
- y2 transposed back to feature-major via the DMA-engine xbar transpose
  (dma_start_transpose): free on the compute engines.
- PE p-state warmup (dummy matmuls) before the stats phase.

If any bias is nonzero the kernel falls back to the generic baseline build.
"""
import numpy as np
import ml_dtypes
from contextlib import ExitStack

import concourse.bacc as bacc
import concourse.mybir as mybir
import concourse.tile as tile
from concourse.bass_utils import run_bass_kernel_spmd

F32 = mybir.dt.float32
F32R = mybir.dt.float32r
BF16 = mybir.dt.bfloat16
F8 = mybir.dt.float8e4
AF = mybir.ActivationFunctionType
ALU = mybir.AluOpType
DR = mybir.MatmulPerfMode.DoubleRow

B, T, C, H, D = 2, 2048, 512, 8, 64
TQ = 512          # query tokens per core
P = 128
KC = C // P       # 4   C partition-chunks
NT = T // 512     # 4   T tiles of 512
NTK = T // P      # 16  key-token chunks of 128
C4 = 4 * C        # 2048
KC4 = C4 // P     # 16
EPS = 1e-5
N_CORES = 8
TH = TQ // 2      # 256  query half

WQ_SCALE = 2.0 ** 13
WK_SCALE = 2.0 ** 10
Q_COPY = 2.0 ** -8
K_COPY = 2.0 ** -5
SIG_SCALE = 2.0 ** -10
RSQRT2 = float(1.0 / np.sqrt(2.0))

_BUILT = {}


def _build_fast():
    nc = bacc.Bacc("TRN2", target_bir_lowering=False, debug=False)

    xT_d = nc.dram_tensor("xT", [NT, P, KC, 512], BF16, kind="ExternalInput")
    coulT_d = nc.dram_tensor("coulT", [NT, P, 4, TQ], BF16, kind="ExternalInput")
    wq8_d = nc.dram_tensor("wq8", [P, KC, C], F8, kind="ExternalInput")
    wk8_d = nc.dram_tensor("wk8", [P, KC, C], F8, kind="ExternalInput")
    wv_d = nc.dram_tensor("wv", [P, KC, C], BF16, kind="ExternalInput")
    wself8_d = nc.dram_tensor("wself8", [P, KC, C], F8, kind="ExternalInput")
    wproj_d = nc.dram_tensor("wproj", [P, KC, C], BF16, kind="ExternalInput")
    wfc_d = nc.dram_tensor("wfc", [P, KC, C4], BF16, kind="ExternalInput")
    wfcp_d = nc.dram_tensor("wfcp", [P, KC4, C], BF16, kind="ExternalInput")
    outT_d = nc.dram_tensor("outT", [P, KC, TQ], F32, kind="ExternalOutput")

    with tile.TileContext(nc) as tc, ExitStack() as octx:
        cstP = octx.enter_context(tc.tile_pool(name="cstP", bufs=1))
        kvP = octx.enter_context(tc.tile_pool(name="kvP", bufs=1))
        wA = octx.enter_context(tc.tile_pool(name="wA", bufs=1))
        wM = octx.enter_context(tc.tile_pool(name="wM", bufs=1))
        rowP = octx.enter_context(tc.tile_pool(name="rowP", bufs=1))
        accP = octx.enter_context(tc.tile_pool(name="accP", bufs=1))

        # ---- constants (memset: no DMA latency) ---------------------------
        cst_sb = cstP.tile([P, 2], BF16)
        cm_neg = cst_sb[:, 0:1]     # -1/C
        cm_pos = cst_sb[:, 1:2]     # +1/C
        nc.vector.memset(cm_neg, -1.0 / C)
        nc.vector.memset(cm_pos, 1.0 / C)
        onesr_sb = cstP.tile([1, P], BF16)
        nc.vector.memset(onesr_sb, 1.0)
        onesrf = cstP.tile([1, P], F32)
        nc.vector.memset(onesrf, 1.0)
        eps1 = cstP.tile([1, 1], F32)
        nc.vector.memset(eps1, EPS)
        one11 = cstP.tile([1, 1], F32)
        nc.vector.memset(one11, 1.0)
        warm_sb = cstP.tile([P, 512], BF16)
        nc.vector.memset(warm_sb, 0.0)
        # preload the sqrt activation table off the LN1 critical chain
        sqrt_pre = cstP.tile([1, 1], F32)
        nc.scalar.activation(sqrt_pre, eps1, AF.Sqrt, bias=eps1)

        with ExitStack() as xctx:
            xP = xctx.enter_context(tc.tile_pool(name="xP", bufs=3))
            x_t = [xP.tile([P, KC, 512], BF16, tag="xt", name=f"xt{n}")
                   for n in range(NT)]
            wq8_sb = wA.tile([P, KC, C], F8)
            wk8_sb = wA.tile([P, KC, C], F8)
            wself8_sb = wA.tile([P, KC, C], F8)
            wv_sb = wA.tile([P, KC, C], BF16)
            wproj_sb = wA.tile([P, KC, C], BF16)
            coulQ = [kvP.tile([P, 4, TQ], BF16, name=f"coulQ{n}")
                     for n in range(NT)]
            for kp in range(0, KC, 2):
                nc.sync.dma_start(x_t[0][:, kp:kp + 2], xT_d[0, :, kp:kp + 2])
            nc.sync.dma_start(wk8_sb, wk8_d[:, :])
            nc.sync.dma_start(wq8_sb, wq8_d[:, :])
            nc.sync.dma_start(x_t[1], xT_d[1])
            for n in range(2, NT):
                nc.sync.dma_start(x_t[n], xT_d[n])
            nc.sync.dma_start(coulQ[0], coulT_d[0])
            # late-needed weights on the SP queue (it idles after this);
            # Pool queue stays free for prologue copies
            wfc_sb = wM.tile([P, KC, C4], BF16)
            wfcp_sb = wM.tile([P, KC4, C], BF16)
            nc.sync.dma_start(wproj_sb, wproj_d[:, :])
            for kp in range(0, KC, 2):
                nc.sync.dma_start(wfc_sb[:, kp:kp + 2], wfc_d[:, kp:kp + 2])
            for kc in range(0, KC4, 8):
                nc.sync.dma_start(wfcp_sb[:, kc:kc + 8], wfcp_d[:, kc:kc + 8])
            nc.gpsimd.dma_start(wself8_sb, wself8_d[:, :])
            nc.gpsimd.dma_start(wv_sb, wv_d[:, :])

            # ---- long-lived activations ----------------------------------
            xc_t = [kvP.tile([P, KC, 512], BF16, name=f"xc{n}") for n in range(NT)]
            xc8_t = [kvP.tile([P, KC, 512], F8, name=f"xc8{n}") for n in range(NT)]
            z8 = kvP.tile([P, KC, TQ], F8)
            # k8 is plain feature-major fp8. The score matmul for head
            # h = 2*chk + e contracts the WHOLE 128-partition chunk in
            # DoubleRow form: the k operand's slab dim is a stride-0
            # broadcast, and the q operand (qe) carries real zeros on the
            # other head's 64 partitions and on slab 1, so only head h's
            # 64 features contribute. All DR outputs sit at partition 0.
            k8 = kvP.tile([P, KC, T], F8)          # (f, chunk, kt)
            qe = kvP.tile([P, 2, 2, KC, TQ], F8)   # (f, slab, e, chk, q)
            v_t = [kvP.tile([P, 4, C], BF16, name=f"v{n}") for n in range(NT)]
            y2T = kvP.tile([P, 4, C], BF16)        # token-major y2 (qh*2+qb, f)
            y2F = kvP.tile([P, KC, TQ], BF16)      # feature-major y2

            rowR = xctx.enter_context(tc.tile_pool(name="rowR", bufs=2))
            attS = xctx.enter_context(tc.tile_pool(name="attS", bufs=3))
            # static zero regions of qe: all of slab 1, and the other
            # head's 64 partitions of slab 0
            nc.vector.memset(qe[:, 1], 0.0)
            nc.gpsimd.memset(qe[64:128, 0, 0], 0.0)
            nc.gpsimd.memset(qe[0:64, 0, 1], 0.0)

            nm_t = [rowR.tile([1, 512], BF16, tag="nm", name=f"nm{n}")
                    for n in range(NT)]
            r_t = [rowR.tile([1, 512], F32, tag="rr", name=f"rr{n}")
                   for n in range(NT)]
            mb_sb = [rowR.tile([P, 512], BF16, tag="mb", name=f"mb{n}")
                     for n in range(NT)]
            rcolv_t = [rowP.tile([P, 4], F32, name=f"rcv{n}") for n in range(NT)]
            rcols_t = [rowP.tile([P, 4], F32, name=f"rcs{n}") for n in range(NT)]
            rs_sb = rowP.tile([P, 512], BF16)

            # ======= K/V/Q producers (full-chunk DoubleRow, dst base 0) ====
            def emit_k8(n, chk, psMM, split=False):
                ps = psMM.tile([P, 512], F32, tag="mm")
                for ci, c0 in enumerate((0, 2)):
                    nc.tensor.matmul(
                        ps,
                        lhsT=wk8_sb[:, c0:c0 + 2, chk * P:(chk + 1) * P],
                        rhs=xc8_t[n][:, c0:c0 + 2, :],
                        start=(ci == 0), stop=(ci == 1), perf_mode=DR)
                dst = k8[:, chk, n * 512:(n + 1) * 512]
                if split:
                    nc.scalar.activation(dst, ps, AF.Copy, scale=K_COPY)
                else:
                    nc.vector.tensor_scalar(dst, ps, K_COPY, None, ALU.mult)

            def emit_q8(chk, psMM):
                ps = psMM.tile([P, 512], F32, tag="mm")
                for ci, c0 in enumerate((0, 2)):
                    nc.tensor.matmul(
                        ps,
                        lhsT=wq8_sb[:, c0:c0 + 2, chk * P:(chk + 1) * P],
                        rhs=z8[:, c0:c0 + 2, :],
                        start=(ci == 0), stop=(ci == 1), perf_mode=DR)
                for e in range(2):
                    dst = qe[64 * e:64 * e + 64, 0, e, chk, :]
                    src = ps[64 * e:64 * e + 64, :]
                    if e == 0:
                        nc.scalar.activation(dst, src, AF.Copy, scale=Q_COPY)
                    else:
                        nc.vector.tensor_scalar(dst, src, Q_COPY, None,
                                                ALU.mult)

            def emit_v(n, c, psMM):
                ps = psMM.tile([P, 512], F32, tag="mm")
                for kc in range(KC):
                    nc.tensor.matmul(ps, lhsT=xc_t[n][:, kc, c * P:(c + 1) * P],
                                     rhs=wv_sb[:, kc], start=(kc == 0),
                                     stop=(kc == KC - 1))
                nc.vector.tensor_scalar(v_t[n][:, c], ps,
                                        rcolv_t[n][:, c:c + 1], None, ALU.mult)

            def stats_tile(n, psST, psRC, psBC):
                xt = x_t[n]
                sq_t = sqP.tile([P, KC, 512], BF16, tag="sq", name=f"sq{n}")
                if n < 2:
                    nc.vector.tensor_tensor(out=sq_t[:, 0:2], in0=xt[:, 0:2],
                                            in1=xt[:, 0:2], op=ALU.mult)
                    nc.scalar.square(sq_t[:, 2:4], xt[:, 2:4])
                else:
                    nc.scalar.square(sq_t, xt)
                ps_m = psST.tile([1, 512], F32, tag="st")
                for kc in range(KC):
                    nc.tensor.matmul(ps_m, lhsT=cm_neg, rhs=xt[:, kc],
                                     start=(kc == 0), stop=(kc == KC - 1))
                nc.scalar.activation(nm_t[n], ps_m, AF.Copy)
                ps_v = psST.tile([1, 512], F32, tag="st")
                for kc in range(KC):
                    nc.tensor.matmul(ps_v, lhsT=cm_pos, rhs=sq_t[:, kc],
                                     start=(kc == 0), stop=(kc == KC - 1))
                msq = srowP.tile([1, 512], F32, tag="row", name=f"msq{n}")
                nc.scalar.square(msq, nm_t[n])
                vrow = srowP.tile([1, 512], F32, tag="row", name=f"vr{n}")
                nc.vector.tensor_tensor(out=vrow, in0=ps_v, in1=msq,
                                        op=ALU.subtract)
                sd = srowP.tile([1, 512], F32, tag="row", name=f"sd{n}")
                nc.scalar.activation(sd, vrow, AF.Sqrt, bias=eps1)
                nc.vector.reciprocal(r_t[n], sd)
                rc_ps = psRC.tile([P, 4], F32, tag="rc", name=f"rc{n}")
                for c in range(4):
                    nc.tensor.matmul(rc_ps[:, c:c + 1],
                                     lhsT=r_t[n][:, c * P:(c + 1) * P],
                                     rhs=one11, is_transpose=True,
                                     start=True, stop=True)
                nc.vector.tensor_copy(rcolv_t[n], rc_ps)
                nc.vector.tensor_scalar(rcols_t[n], rc_ps, SIG_SCALE, None,
                                        ALU.mult)
                mbn = psBC.tile([P, 512], F32, tag="bc", name=f"mbn{n}")
                nc.tensor.matmul(mbn, lhsT=onesr_sb, rhs=nm_t[n],
                                 start=True, stop=True)
                nc.scalar.activation(mb_sb[n], mbn, AF.Copy)
                for kp in range(0, KC, 2):
                    nc.vector.tensor_tensor(
                        out=xc_t[n][:, kp:kp + 2], in0=xt[:, kp:kp + 2],
                        in1=mb_sb[n][:, None, :].to_broadcast([P, 2, 512]),
                        op=ALU.add)
                for kp in range(0, KC, 2):
                    if n == 0:
                        nc.vector.tensor_copy(xc8_t[n][:, kp:kp + 2],
                                              xc_t[n][:, kp:kp + 2])
                    else:
                        nc.gpsimd.tensor_copy(xc8_t[n][:, kp:kp + 2],
                                              xc_t[n][:, kp:kp + 2])
                if n > 0:
                    nc.gpsimd.dma_start(coulQ[n], coulT_d[n])

            # ======= Phase 1: warmup + stats + tile-0 K/V/Q ================
            mmS = ExitStack()
            psMM = mmS.enter_context(tc.tile_pool(name="psMM", bufs=2,
                                                  space="PSUM", side="right"))
            with tc.tile_pool(name="sqP", bufs=2) as sqP, \
                 tc.tile_pool(name="srowP", bufs=2) as srowP, \
                 tc.tile_pool(name="psST", bufs=2, space="PSUM") as psST, \
                 tc.tile_pool(name="psRC", bufs=1, space="PSUM") as psRC, \
                 tc.tile_pool(name="psBC", bufs=2, space="PSUM") as psBC, \
                 tc.tile_pool(name="psW", bufs=1, space="PSUM") as psW:
                ps_w = psW.tile([1, 512], F32)
                for _ in range(8):
                    nc.tensor.matmul(ps_w, lhsT=cm_neg, rhs=warm_sb,
                                     start=True, stop=True)
                stats_tile(0, psST, psRC, psBC)
                rs_ps = psBC.tile([P, 512], F32, tag="bc", name="rs0")
                nc.tensor.matmul(rs_ps, lhsT=onesrf, rhs=r_t[0],
                                 start=True, stop=True)
                nc.scalar.activation(rs_sb, rs_ps, AF.Copy)
                for kp in range(0, KC, 2):
                    nc.vector.tensor_tensor(
                        out=z8[:, kp:kp + 2], in0=xc_t[0][:, kp:kp + 2],
                        in1=rs_sb[:, None, :].to_broadcast([P, 2, 512]),
                        op=ALU.mult)
                for chk in range(4):
                    emit_q8(chk, psMM)
                    emit_k8(0, chk, psMM, split=(chk % 2 == 0))
                for c in range(4):
                    emit_v(0, c, psMM)
                stats_tile(1, psST, psRC, psBC)
                for chk in range(4):
                    emit_k8(1, chk, psMM, split=(chk % 2 == 0))
                for c in range(4):
                    emit_v(1, c, psMM)
                stats_tile(2, psST, psRC, psBC)
                stats_tile(3, psST, psRC, psBC)

            # ======= Attention + interleaved first-half epilogue ===========
            y3_sb = accP.tile([P, KC, TQ], BF16, tag="y3")
            z2_sb = accP.tile([P, KC, TQ], BF16, tag="z2")
            g_sb = accP.tile([P, KC4, TQ], BF16, tag="g")
            out_sb = accP.tile([P, KC, TQ], F32, tag="out")
            SL = [slice(0, TH), slice(TH, TQ)]

            def emit_self(qh, qb, y_bank):
                for ci, c0 in enumerate((0, 2)):
                    nc.tensor.matmul(
                        y_bank,
                        lhsT=z8[:, c0:c0 + 2,
                                (2 * qh + qb) * P:(2 * qh + qb + 1) * P],
                        rhs=wself8_sb[:, c0:c0 + 2, :],
                        start=(ci == 0), stop=False, perf_mode=DR)

            def emit_scores(sc, heads, tkc, qh):
                for i, h in enumerate(heads):
                    chk, e = h // 2, h % 2
                    kslab = k8[:, chk, tkc * P:(tkc + 1) * P]
                    nc.tensor.matmul(
                        sc[:, i, :],
                        lhsT=kslab[:, None, :].to_broadcast([P, 2, P]),
                        rhs=qe[:, :, e, chk, qh * TH:(qh + 1) * TH],
                        start=True, stop=True, perf_mode=DR)

            def emit_tail(s_t, heads, tkc, qh, y_banks):
                n, j = tkc // 4, tkc % 4
                nc.vector.tensor_tensor(
                    out=s_t, in0=s_t,
                    in1=coulQ[n][:, j, qh * TH:(qh + 1) * TH][:, None, :]
                        .to_broadcast([P, len(heads), TH]),
                    op=ALU.mult)
                for i, h in enumerate(heads):
                    for qb in range(2):
                        nc.tensor.matmul(
                            y_banks[qb][:, 64 * h:64 * h + 64],
                            lhsT=s_t[:, i, qb * P:(qb + 1) * P],
                            rhs=v_t[n][:, j, 64 * h:64 * h + 64],
                            start=False, stop=(tkc == NTK - 1))

            if True:
                yS = ExitStack()
                psYP = yS.enter_context(
                    tc.tile_pool(name="psY", bufs=1, space="PSUM"))

                # ---------------- pass 0 (query half 0) -------------------
                y_banks = [psYP.tile([P, TQ], F32, tag=f"y{qb}",
                                     name=f"y_0_{qb}") for qb in range(2)]
                with tc.tile_pool(name="psSC0", bufs=2, space="PSUM") as psSC0:
                    emit_self(0, 0, y_banks[0])
                    emit_self(0, 1, y_banks[1])
                    pend = None
                    for n in range(NT):
                        for s in range(8):
                            hh, ti = s // 4, s % 4
                            tkc = 4 * n + ti
                            if n < NT - 2:
                                # tiles 0/1 made in the prologue; 2/3 here
                                if s < 4:
                                    emit_k8(n + 2, s, psMM)
                                else:
                                    emit_v(n + 2, s - 4, psMM)
                            heads = range(4 * hh, 4 * hh + 4)
                            sc = psSC0.tile([P, 4, TH], F32, tag="sc")
                            emit_scores(sc, heads, tkc, 0)
                            s_t = attS.tile([P, 4, TH], BF16, tag="st",
                                            name=f"st0_{hh}_{tkc}")
                            nc.scalar.activation(
                                s_t, sc, AF.Sigmoid,
                                scale=rcols_t[n][:, ti:ti + 1])
                            if pend is not None:
                                emit_tail(*pend)
                            pend = (s_t, heads, tkc, 0, y_banks)
                    emit_tail(*pend)
                mmS.close()
                nc.scalar.activation(y2T[:, 0, :], y_banks[0], AF.Copy)
                nc.vector.tensor_copy(y2T[:, 1, :], y_banks[1])
                for qb in range(2):
                    nc.sync.dma_start_transpose(
                        y2F[:, :, qb * P:(qb + 1) * P], y2T[:, qb, :])

                # -------- pass 1 (query half 1) + epilogue for half 0 ------
                y_banks = [psYP.tile([P, TQ], F32, tag=f"y{qb}",
                                     name=f"y_1_{qb}") for qb in range(2)]
                ln2row = {}

                def epi_proj(hf, psP5, p5tag="e1"):
                    sl = SL[hf]
                    for j in range(KC):
                        psf = psP5.tile([P, 2, TH], F32, tag=p5tag,
                                        name=f"p5_{hf}_{j}")
                        ps = psf[:, 0, :]
                        for kc in range(KC):
                            nc.tensor.matmul(
                                ps, lhsT=wproj_sb[:, kc, j * P:(j + 1) * P],
                                rhs=y2F[:, kc, sl],
                                start=(kc == 0), stop=(kc == KC - 1))
                        if j % 2 == 0:
                            nc.vector.tensor_copy(y3_sb[:, j, sl], ps)
                        else:
                            nc.scalar.activation(y3_sb[:, j, sl], ps, AF.Copy)

                def epi_ln2_stats(hf, psST2, st2tag="st2"):
                    sl = SL[hf]
                    y3h = y3_sb[:, :, sl]
                    sq2 = ln2S.tile([P, KC, TH], BF16, tag="sq2",
                                    name=f"sq2_{hf}")
                    nc.vector.tensor_tensor(out=sq2, in0=y3h, in1=y3h,
                                            op=ALU.mult)
                    st2 = psST2.tile([1, 2, TH], F32, tag=st2tag,
                                     name=f"st2_{hf}")
                    ps_m2 = st2[:, 0, :]
                    for kc in range(KC):
                        nc.tensor.matmul(ps_m2, lhsT=cm_neg, rhs=y3h[:, kc],
                                         start=(kc == 0), stop=(kc == KC - 1))
                    nm2 = ln2R.tile([1, TH], BF16, tag="row2", name=f"nm2_{hf}")
                    nc.scalar.activation(nm2, ps_m2, AF.Copy)
                    ps_v2 = st2[:, 1, :]
                    for kc in range(KC):
                        nc.tensor.matmul(ps_v2, lhsT=cm_pos, rhs=sq2[:, kc],
                                         start=(kc == 0), stop=(kc == KC - 1))
                    msq2 = ln2R.tile([1, TH], F32, tag="row2",
                                     name=f"msq2_{hf}")
                    nc.vector.tensor_tensor(out=msq2, in0=nm2, in1=nm2,
                                            op=ALU.mult)
                    v2 = ln2R.tile([1, TH], F32, tag="row2", name=f"v2_{hf}")
                    nc.vector.tensor_tensor(out=v2, in0=ps_v2, in1=msq2,
                                            op=ALU.subtract)
                    sd2 = ln2R.tile([1, TH], F32, tag="row2", name=f"sd2_{hf}")
                    nc.scalar.activation(sd2, v2, AF.Sqrt, bias=eps1)
                    r2 = ln2R.tile([1, TH], F32, tag="row2", name=f"r2_{hf}")
                    nc.vector.reciprocal(r2, sd2)
                    ln2row[hf] = (nm2, r2)

                def epi_z2(hf, psBC2, bc2tag="bc2"):
                    sl = SL[hf]
                    y3h = y3_sb[:, :, sl]
                    nm2, r2 = ln2row[hf]
                    bc2 = psBC2.tile([P, 2, TH], F32, tag=bc2tag,
                                     name=f"bc2_{hf}")
                    mb2 = bc2[:, 0, :]
                    nc.tensor.matmul(mb2, lhsT=onesr_sb, rhs=nm2,
                                     start=True, stop=True)
                    rs2 = bc2[:, 1, :]
                    nc.tensor.matmul(rs2, lhsT=onesrf, rhs=r2,
                                     start=True, stop=True)
                    for kp in range(0, KC, 2):
                        nc.vector.tensor_tensor(
                            out=z2_sb[:, kp:kp + 2, sl], in0=y3h[:, kp:kp + 2],
                            in1=mb2[:, None, :].to_broadcast([P, 2, TH]),
                            op=ALU.add)
                    for kp in range(0, KC, 2):
                        z2p = z2_sb[:, kp:kp + 2, sl]
                        nc.vector.tensor_tensor(
                            out=z2p, in0=z2p,
                            in1=rs2[:, None, :].to_broadcast([P, 2, TH]),
                            op=ALU.mult)

                def epi_fc(hf, mo, psMLP):
                    sl = SL[hf]
                    ps = psMLP.tile([P, TH], F32, tag="fc")
                    for kc in range(KC):
                        nc.tensor.matmul(
                            ps, lhsT=wfc_sb[:, kc, mo * P:(mo + 1) * P],
                            rhs=z2_sb[:, kc, sl],
                            start=(kc == 0), stop=(kc == KC - 1))
                    # gelu = x * (1 + erf(x/sqrt2)); the 0.5 is folded into
                    # wfcp host-side. erf shares the sigmoid act table.
                    er = attS.tile([P, TH], BF16, tag="er", name=f"er{hf}_{mo}")
                    nc.scalar.activation(er, ps, AF.Erf, scale=RSQRT2)
                    nc.vector.scalar_tensor_tensor(
                        out=g_sb[:, mo, sl], in0=er, scalar=1.0, in1=ps,
                        op0=ALU.add, op1=ALU.mult)

                def epi_fcp(hf, mo, oj):
                    sl = SL[hf]
                    for j in range(KC):
                        nc.tensor.matmul(
                            oj[j], lhsT=wfcp_sb[:, mo, j * P:(j + 1) * P],
                            rhs=g_sb[:, mo, sl],
                            start=(mo == 0), stop=(mo == KC4 - 1))

                def epi_out(hf, oj):
                    sl = SL[hf]
                    for j in range(KC):
                        if j % 2 == 0:
                            nc.vector.tensor_copy(out_sb[:, j, sl], oj[j])
                            nc.gpsimd.dma_start(outT_d[:, j, sl],
                                                out_sb[:, j, sl])
                        else:
                            nc.scalar.activation(out_sb[:, j, sl], oj[j],
                                                 AF.Copy)
                            nc.sync.dma_start(outT_d[:, j, sl],
                                              out_sb[:, j, sl])

                with tc.tile_pool(name="ln2R", bufs=8) as ln2R, \
                     tc.tile_pool(name="ln2S", bufs=2) as ln2S:
                    sc1S = ExitStack()
                    psSC1 = sc1S.enter_context(
                        tc.tile_pool(name="psSC1", bufs=2, space="PSUM"))
                    epiA = ExitStack()
                    # one bank serves proj (as [:,0,:] slices) and then bc2
                    psE1 = epiA.enter_context(
                        tc.tile_pool(name="psE1", bufs=1, space="PSUM"))
                    psST2a = epiA.enter_context(
                        tc.tile_pool(name="psST2a", bufs=1, space="PSUM"))

                    emit_self(1, 0, y_banks[0])
                    emit_self(1, 1, y_banks[1])
                    ustep = 0
                    pend = None
                    for n in range(NT):
                        for s in range(8):
                            hh, ti = s // 4, s % 4
                            tkc = 4 * n + ti
                            heads = range(4 * hh, 4 * hh + 4)
                            sc = psSC1.tile([P, 4, TH], F32, tag="sc1")
                            emit_scores(sc, heads, tkc, 1)
                            s_t = attS.tile([P, 4, TH], BF16, tag="st",
                                            name=f"st1_{hh}_{tkc}")
                            nc.scalar.activation(
                                s_t, sc, AF.Sigmoid,
                                scale=rcols_t[n][:, ti:ti + 1])
                            if pend is not None:
                                emit_tail(*pend)
                            pend = (s_t, heads, tkc, 1, y_banks)
                            # ---- feed half-0 proj/LN2 into this window
                            if ustep == 3:
                                epi_proj(0, psE1)
                            elif ustep == 8:
                                epi_ln2_stats(0, psST2a)
                            elif ustep == 12:
                                epi_z2(0, psE1, bc2tag="e1")
                            ustep += 1
                    emit_tail(*pend)
                    nc.scalar.activation(y2T[:, 2, :], y_banks[0], AF.Copy)
                    nc.vector.tensor_copy(y2T[:, 3, :], y_banks[1])
                    for qb in range(2):
                        nc.sync.dma_start_transpose(
                            y2F[:, :, TH + qb * P:TH + (qb + 1) * P],
                            y2T[:, 2 + qb, :])
                    epiA.close()
                    sc1S.close()
                    yS.close()

                    # ---- tail: MLP half 0 + full epilogue half 1, deep-
                    # pipelined; fresh PSUM pools (6 banks) -----------------
                    with tc.tile_pool(name="psTL", bufs=1,
                                      space="PSUM") as psTL, \
                         tc.tile_pool(name="psST2b", bufs=1,
                                      space="PSUM") as psST2b, \
                         tc.tile_pool(name="psMLP", bufs=2,
                                      space="PSUM") as psMLP, \
                         tc.tile_pool(name="psOJ", bufs=1,
                                      space="PSUM") as psOJ:
                        ojp = [psOJ.tile([P, 2, TH], F32, tag=f"oj{j}",
                                         name=f"oj{j}_h0") for j in range(KC)]
                        oj0 = [ojp[j][:, 0, :] for j in range(KC)]
                        epi_fc(0, 0, psMLP)
                        epi_fc(0, 1, psMLP)
                        epi_proj(1, psTL, p5tag="e1")
                        epi_fc(0, 2, psMLP)
                        epi_fcp(0, 0, oj0)
                        epi_ln2_stats(1, psST2b)
                        epi_fc(0, 3, psMLP)
                        epi_fcp(0, 1, oj0)
                        epi_z2(1, psTL, bc2tag="e1")
                        for mo in range(4, KC4 + 2):
                            if mo < KC4:
                                epi_fc(0, mo, psMLP)
                            epi_fcp(0, mo - 2, oj0)
                        epi_out(0, oj0)
                        ojp1 = [psOJ.tile([P, 2, TH], F32, tag=f"oj{j}",
                                          name=f"oj{j}_h1")
                                for j in range(KC)]
                        oj1 = [ojp1[j][:, 0, :] for j in range(KC)]
                        for mo in range(KC4 + 2):
                            if mo < KC4:
                                epi_fc(1, mo, psMLP)
                            if mo >= 2:
                                epi_fcp(1, mo - 2, oj1)
                        epi_out(1, oj1)

    nc.compile()
    return nc


def _fmt_lhs(w):
    """[Cin, Cout] -> [128, Cin//128, Cout] partition-major lhsT layout."""
    return np.ascontiguousarray(
        w.reshape(w.shape[0] // P, P, w.shape[1]).transpose(1, 0, 2))


def _prep_fast(inputs):
    f32 = np.float32
    f8 = ml_dtypes.float8_e4m3
    x = np.asarray(inputs["x"], f32)
    coul = np.asarray(inputs["coulomb_matrix"], f32)
    g1 = np.asarray(inputs["ln1_g"], f32)
    g2 = np.asarray(inputs["ln2_g"], f32)
    wattn = np.asarray(inputs["w_attn"], f32)
    w_self = np.asarray(inputs["w_self"], f32)
    w_proj = np.asarray(inputs["w_proj"], f32)
    w_fc = np.asarray(inputs["w_fc"], f32)
    w_fcp = np.asarray(inputs["w_fc_proj"], f32)

    wq, wk, wv = wattn[:, 0:C], wattn[:, C:2 * C], wattn[:, 2 * C:]
    wq_f = g1[:, None] * wq * (1.0 / np.sqrt(D))   # score scale folded in
    wk_f = g1[:, None] * wk
    wv_f = g1[:, None] * wv
    shared = {
        "wq8": _fmt_lhs(wq_f * WQ_SCALE).astype(f8),
        "wk8": _fmt_lhs(wk_f * WK_SCALE).astype(f8),
        "wv": _fmt_lhs(wv_f).astype(ml_dtypes.bfloat16),
        "wself8": _fmt_lhs(g1[:, None] * w_self).astype(f8),
        "wproj": _fmt_lhs(w_proj).astype(ml_dtypes.bfloat16),
        "wfc": _fmt_lhs(g2[:, None] * w_fc).astype(ml_dtypes.bfloat16),
        # 0.5 from the erf-form gelu is folded in here
        "wfcp": _fmt_lhs(0.5 * w_fcp).astype(ml_dtypes.bfloat16),
    }
    in_maps = []
    for core in range(N_CORES):
        b, tqi = divmod(core, 4)
        tq0 = tqi * TQ
        xr = np.roll(x[b], -tq0, axis=0)                      # [T, C]
        xT = np.ascontiguousarray(
            xr.T.reshape(KC, P, T).transpose(1, 0, 2)).astype(
                ml_dtypes.bfloat16)                           # [P, KC, T]
        xTt = np.ascontiguousarray(
            xT.reshape(P, KC, NT, 512).transpose(2, 0, 1, 3))  # [NT, P, KC, 512]
        cr = np.roll(coul[b], -tq0, axis=1)[tq0:tq0 + TQ, :]  # [TQ, T]
        coulT = np.ascontiguousarray(
            cr.T.reshape(NT, 4, P, TQ).transpose(0, 2, 1, 3)).astype(
                ml_dtypes.bfloat16)                           # [NT, P, 4, TQ]
        m = dict(shared)
        m["xT"] = xTt
        m["coulT"] = coulT
        in_maps.append(m)
    return in_maps


def _assemble(results):
    out = np.empty((B, T, C), np.float32)
    for core in range(N_CORES):
        b, tqi = divmod(core, 4)
        tq0 = tqi * TQ
        r = results[core]["outT"]                  # [P, KC, TQ]
        o = r.transpose(1, 0, 2).reshape(C, TQ).T  # [TQ, C]
        out[b, tq0:tq0 + TQ] = o
    return out


def _biases_zero(inputs):
    for k in ("b_attn", "b_self", "b_proj", "b_fc", "b_fc_proj",
              "ln1_b", "ln2_b"):
        if np.any(np.asarray(inputs[k], np.float32)):
            return False
    return True


def _get_nc(fast):
    key = "fast" if fast else "generic"
    if key not in _BUILT:
        _BUILT[key] = _build_fast() if fast else _build_generic()
    return _BUILT[key]


def _run(inputs, trace=False):
    fast = _biases_zero(inputs)
    nc = _get_nc(fast)
    in_maps = _prep_fast(inputs) if fast else _prep_generic(inputs)
    res = run_bass_kernel_spmd(nc, in_maps, core_ids=list(range(N_CORES)),
                               trace=trace)
    return _assemble(res.results), res


def kernel(**inputs):
    out, _ = _run(inputs)
    return out


# ===== generic fallback (nonzero biases): original baseline =====
def _build_generic():
    nc = bacc.Bacc("TRN2", target_bir_lowering=False, debug=False)

    xT_d = nc.dram_tensor("xT", [P, KC, T], BF16, kind="ExternalInput")
    coulT_d = nc.dram_tensor("coulT", [NTK, P, TQ], BF16, kind="ExternalInput")
    wq_d = nc.dram_tensor("wq", [P, KC, C], BF16, kind="ExternalInput")
    wk_d = nc.dram_tensor("wk", [P, KC, C], BF16, kind="ExternalInput")
    wv_d = nc.dram_tensor("wv", [P, KC, C], BF16, kind="ExternalInput")
    wself_d = nc.dram_tensor("wself", [P, KC, C], BF16, kind="ExternalInput")
    wproj_d = nc.dram_tensor("wproj", [P, KC, C], BF16, kind="ExternalInput")
    wfc_d = nc.dram_tensor("wfc", [P, KC, C4], BF16, kind="ExternalInput")
    wfcp_d = nc.dram_tensor("wfcp", [P, KC4, C], BF16, kind="ExternalInput")
    bq_d = nc.dram_tensor("bq", [P, KC], F32, kind="ExternalInput")
    bk_d = nc.dram_tensor("bk", [P, KC], F32, kind="ExternalInput")
    bv_d = nc.dram_tensor("bv", [1, C], F32R, kind="ExternalInput")
    bself_d = nc.dram_tensor("bself", [P, KC], F32, kind="ExternalInput")
    bproj_d = nc.dram_tensor("bproj", [P, KC], F32, kind="ExternalInput")
    bfc_d = nc.dram_tensor("bfc", [P, KC4], F32, kind="ExternalInput")
    bfcp_d = nc.dram_tensor("bfcp", [P, KC], F32, kind="ExternalInput")
    cst_d = nc.dram_tensor("cst", [P, 2], BF16, kind="ExternalInput")  # [1, 1/C]
    onesr_d = nc.dram_tensor("onesr", [1, P], F32R, kind="ExternalInput")
    outT_d = nc.dram_tensor("outT", [P, KC, TQ], F32, kind="ExternalOutput")

    with tile.TileContext(nc) as tc, ExitStack() as octx:
        cst = octx.enter_context(tc.tile_pool(name="cst", bufs=1))
        lateP = octx.enter_context(tc.tile_pool(name="lateP", bufs=1))
        wfcP = octx.enter_context(tc.tile_pool(name="wfcP", bufs=1))
        wB = octx.enter_context(tc.tile_pool(name="wB", bufs=1))
        zP = octx.enter_context(tc.tile_pool(name="zP", bufs=1))
        qkvP = octx.enter_context(tc.tile_pool(name="qkvP", bufs=1))

        # ---- x tiles stream in first (16 x 256KB on the sync queue) --------
        z_sb = zP.tile([P, KC, T], BF16)
        q_sb = qkvP.tile([P, KC, TQ], BF16)
        k_sb = qkvP.tile([P, KC, T], BF16)
        v_sb = qkvP.tile([P, NTK, C], BF16)

        # ---- constants / biases (vector queue) -----------------------------
        cst_sb = cst.tile([P, 2], BF16)
        nc.sync.dma_start(cst_sb, cst_d[:, :])
        ones_col = cst_sb[:, 0:1]
        cm_col = cst_sb[:, 1:2]
        onesr_sb = cst.tile([1, P], F32R)
        nc.sync.dma_start(onesr_sb, onesr_d[:, :])
        eps1 = cst.tile([1, 1], F32)
        nc.vector.memset(eps1, EPS)
        bq_sb = cst.tile([P, KC], F32)
        bk_sb = cst.tile([P, KC], F32)
        bself_sb = cst.tile([P, KC], F32)
        bproj_sb = cst.tile([P, KC], F32)
        bfc_sb = cst.tile([P, KC4], F32)
        bfcp_sb = cst.tile([P, KC], F32)
        bv_sb = cst.tile([1, C], F32R)

        # ---- weights: scalar queue for attention-side, gpsimd for MLP ------
        wself_sb = wB.tile([P, KC, C], BF16)
        wproj_sb = wB.tile([P, KC, C], BF16)
        wfc_sb = wfcP.tile([P, KC, C4], BF16)
        wfcp_sb = wfcP.tile([P, KC4, C], BF16)
        for kc in range(KC):
            nc.gpsimd.dma_start(wfc_sb[:, kc], wfc_d[:, kc])
        for kc in range(0, KC4, 4):
            nc.gpsimd.dma_start(wfcp_sb[:, kc:kc + 4], wfcp_d[:, kc:kc + 4])
        for sb, d in ((wself_sb, wself_d), (wproj_sb, wproj_d)):
            for kc in range(KC):
                nc.gpsimd.dma_start(sb[:, kc], d[:, kc])

        with ExitStack() as actx:
            wA = actx.enter_context(tc.tile_pool(name="wA", bufs=1))
            wq_sb = wA.tile([P, KC, C], BF16)
            wk_sb = wA.tile([P, KC, C], BF16)
            wv_sb = wA.tile([P, KC, C], BF16)
            for sb, d in ((wq_sb, wq_d), (wk_sb, wk_d), (wv_sb, wv_d)):
                for kc in range(KC):
                    nc.gpsimd.dma_start(sb[:, kc], d[:, kc])

            # ======= Phase 1: LayerNorm 1, pipelined per 512-token tile =====
            # broadcasts of per-token mean/rstd are K=1 matmuls into PSUM;
            # the DVE applies read the PSUM operand directly.
            with tc.tile_pool(name="lnX", bufs=6) as lnX, \
                 tc.tile_pool(name="lnR", bufs=8) as lnR, \
                 tc.tile_pool(name="lnS", bufs=4) as lnS, \
                 tc.tile_pool(name="psLN", bufs=2, space="PSUM") as psLN, \
                 tc.tile_pool(name="psMM", bufs=2, space="PSUM") as psMM:
                x_tiles = {}
                for n in range(NT):
                    xt = lnX.tile([P, KC, 512], BF16, tag="xt", name=f"xt_{n}")
                    nc.sync.dma_start(xt, xT_d[:, :, n * 512:(n + 1) * 512])
                    x_tiles[n] = xt
                for sb, d in ((bq_sb, bq_d), (bk_sb, bk_d), (bself_sb, bself_d),
                              (bproj_sb, bproj_d), (bfc_sb, bfc_d), (bfcp_sb, bfcp_d)):
                    nc.sync.dma_start(sb, d[:, :])
                nc.sync.dma_start(bv_sb, bv_d[:, :])
                for n in range(NT):
                    sl = slice(n * 512, (n + 1) * 512)
                    xt = x_tiles[n]
                    # x^2 on ACT runs in parallel with the mean matmuls;
                    # uncentered variance: var = E[x^2] - mean^2 (row math).
                    sq_t = lnS.tile([P, KC, 512], BF16, tag="sq", name=f"sq{n}")
                    nc.scalar.square(sq_t, xt)
                    ps_m = psLN.tile([1, 512], F32, tag="st")
                    for kc in range(KC):
                        nc.tensor.matmul(ps_m, lhsT=cm_col, rhs=xt[:, kc],
                                         start=(kc == 0), stop=(kc == KC - 1))
                    m_row = lnR.tile([1, 512], F32R, tag="row", name=f"mrow{n}")
                    nc.scalar.activation(m_row, ps_m, AF.Copy)
                    mb_ps = psLN.tile([P, 512], F32, tag="mbp", name=f"mbp{n}")
                    nc.tensor.matmul(mb_ps, lhsT=onesr_sb, rhs=m_row,
                                     start=True, stop=True)
                    ps_v = psLN.tile([1, 512], F32, tag="st")
                    for kc in range(KC):
                        nc.tensor.matmul(ps_v, lhsT=cm_col, rhs=sq_t[:, kc],
                                         start=(kc == 0), stop=(kc == KC - 1))
                    msq_row = lnR.tile([1, 512], F32, tag="row", name=f"msqrow{n}")
                    nc.scalar.square(msq_row, m_row.bitcast(F32))
                    v_row = lnR.tile([1, 512], F32, tag="row", name=f"vrow{n}")
                    nc.vector.tensor_tensor(out=v_row, in0=ps_v, in1=msq_row,
                                            op=ALU.subtract)
                    nc.vector.tensor_tensor(
                        out=z_sb[:, :, sl], in0=xt,
                        in1=mb_ps[:, None, :].to_broadcast([P, KC, 512]),
                        op=ALU.subtract)
                    lnr = lnR.tile([1, 512], F32, tag="row", name=f"lnrow{n}")
                    nc.scalar.activation(lnr, v_row, AF.Ln, bias=eps1)
                    rs_row = lnR.tile([1, 512], F32R, tag="row", name=f"rsrow{n}")
                    nc.scalar.activation(rs_row, lnr, AF.Exp, scale=-0.5)
                    rsb_ps = psLN.tile([P, 512], F32, tag="rsp", name=f"rsp{n}")
                    nc.tensor.matmul(rsb_ps, lhsT=onesr_sb, rhs=rs_row,
                                     start=True, stop=True)
                    nc.vector.tensor_tensor(
                        out=z_sb[:, :, sl], in0=z_sb[:, :, sl],
                        in1=rsb_ps[:, None, :].to_broadcast([P, KC, 512]),
                        op=ALU.mult)

                    # ---- q/k/v projections for this token tile ----
                    if n == 0:
                        for mo in range(KC):
                            ps = psMM.tile([P, 512], F32, tag="mm")
                            for kc in range(KC):
                                nc.tensor.matmul(
                                    ps, lhsT=wq_sb[:, kc, mo * P:(mo + 1) * P],
                                    rhs=z_sb[:, kc, 0:TQ],
                                    start=(kc == 0), stop=(kc == KC - 1))
                            if mo < 2:
                                nc.scalar.activation(q_sb[:, mo], ps, AF.Identity,
                                                     bias=bq_sb[:, mo:mo + 1])
                            else:
                                nc.vector.tensor_scalar(q_sb[:, mo], ps,
                                                        bq_sb[:, mo:mo + 1],
                                                        None, ALU.add)
                    for mo in range(KC):
                        ps = psMM.tile([P, 512], F32, tag="mm")
                        for kc in range(KC):
                            nc.tensor.matmul(
                                ps, lhsT=wk_sb[:, kc, mo * P:(mo + 1) * P],
                                rhs=z_sb[:, kc, sl],
                                start=(kc == 0), stop=(kc == KC - 1))
                        if mo < 2:
                            nc.scalar.activation(k_sb[:, mo, sl], ps, AF.Identity,
                                                 bias=bk_sb[:, mo:mo + 1])
                        else:
                            nc.vector.tensor_scalar(k_sb[:, mo, sl], ps,
                                                    bk_sb[:, mo:mo + 1],
                                                    None, ALU.add)
                    for ts_ in range(4 * n, 4 * n + 4):
                        ps = psMM.tile([P, 512], F32, tag="mm")
                        for kc in range(KC):
                            nc.tensor.matmul(ps,
                                             lhsT=z_sb[:, kc, ts_ * P:(ts_ + 1) * P],
                                             rhs=wv_sb[:, kc],
                                             start=(kc == 0), stop=False)
                        nc.tensor.matmul(ps, lhsT=onesr_sb, rhs=bv_sb,
                                         start=False, stop=True)
                        nc.vector.tensor_copy(v_sb[:, ts_], ps)

        # ======= Phase 3: attention (quarter-pipelined scores/sigmoid) ======
        with tc.tile_pool(name="attS", bufs=3) as attS, \
             tc.tile_pool(name="attC", bufs=4) as attC, \
             tc.tile_pool(name="psATT", bufs=1, space="PSUM") as psATT, \
             tc.tile_pool(name="psSC", bufs=2, space="PSUM") as psSC:
            y_ps = [psATT.tile([P, TQ], F32, tag=f"y{j}", name=f"y_ps{j}")
                    for j in range(KC)]
            for j in range(KC):
                for kc in range(KC):
                    nc.tensor.matmul(y_ps[j],
                                     lhsT=wself_sb[:, kc, j * P:(j + 1) * P],
                                     rhs=z_sb[:, kc, 0:TQ],
                                     start=(kc == 0), stop=False)
            for tkc in range(NTK):
                coul_t = attC.tile([P, TQ], BF16, tag="coul")
                nc.sync.dma_start(coul_t, coulT_d[tkc])
                for half in range(2):
                    s_t = attS.tile([P, 4, TQ], BF16, tag="st")
                    for quarter in range(2):
                        sc_ps = psSC.tile([P, 2, TQ], F32, tag="sc")
                        for hh in range(2):
                            h = half * 4 + quarter * 2 + hh
                            chk, po = h // 2, 64 * (h % 2)
                            nc.tensor.matmul(
                                sc_ps[:, hh, :],
                                lhsT=k_sb[po:po + 64, chk, tkc * P:(tkc + 1) * P],
                                rhs=q_sb[po:po + 64, chk, :],
                                start=True, stop=True)
                        nc.scalar.activation(s_t[:, quarter * 2:quarter * 2 + 2, :],
                                             sc_ps, AF.Sigmoid, scale=0.125)
                    nc.vector.tensor_tensor(
                        out=s_t, in0=s_t,
                        in1=coul_t[:, None, :].to_broadcast([P, 4, TQ]),
                        op=ALU.mult)
                    for hh in range(4):
                        h = half * 4 + hh
                        j, po = h // 2, 64 * (h % 2)
                        nc.tensor.matmul(
                            y_ps[j][po:po + 64, :],
                            lhsT=v_sb[:, tkc, 64 * h:64 * h + 64],
                            rhs=s_t[:, hh, :],
                            start=False, stop=(tkc == NTK - 1),
                            tile_position=(0, po))

            # ======= Phase 4: y2 = attention + self + bias ==================
            y2_sb = lateP.tile([P, KC, TQ], BF16, tag="mid_a")
            for j in range(KC):
                if j < 2:
                    nc.vector.tensor_scalar(y2_sb[:, j], y_ps[j],
                                            bself_sb[:, j:j + 1], None, ALU.add)
                else:
                    nc.scalar.activation(y2_sb[:, j], y_ps[j], AF.Identity,
                                         bias=bself_sb[:, j:j + 1])

        # ======= Phase 5: out-proj ==========================================
        y3_sb = lateP.tile([P, KC, TQ], BF16, tag="mid_b")
        with tc.tile_pool(name="psP5", bufs=2, space="PSUM") as psP5:
            for j in range(KC):
                ps = psP5.tile([P, 512], F32, tag="mm")
                for kc in range(KC):
                    nc.tensor.matmul(ps, lhsT=wproj_sb[:, kc, j * P:(j + 1) * P],
                                     rhs=y2_sb[:, kc],
                                     start=(kc == 0), stop=(kc == KC - 1))
                if j % 2 == 0:
                    nc.vector.tensor_scalar(y3_sb[:, j], ps, bproj_sb[:, j:j + 1],
                                            None, ALU.add)
                else:
                    nc.scalar.activation(y3_sb[:, j], ps, AF.Identity,
                                         bias=bproj_sb[:, j:j + 1])

        # ======= Phase 6: LayerNorm 2 (TQ tokens, bf16 out) =================
        z2_sb = lateP.tile([P, KC, TQ], BF16, tag="z2")
        with tc.tile_pool(name="ln2R", bufs=6) as ln2R, \
             tc.tile_pool(name="ln2S", bufs=1) as ln2S, \
             tc.tile_pool(name="ln2T", bufs=4) as ln2T, \
             tc.tile_pool(name="psLN2", bufs=2, space="PSUM") as psLN2:
            sq2 = ln2S.tile([P, KC, 512], BF16, tag="sq2")
            nc.scalar.square(sq2, y3_sb)
            ps_m2 = psLN2.tile([1, 512], F32, tag="st2")
            for kc in range(KC):
                nc.tensor.matmul(ps_m2, lhsT=cm_col, rhs=y3_sb[:, kc],
                                 start=(kc == 0), stop=(kc == KC - 1))
            m2_row = ln2R.tile([1, TQ], F32R, tag="row2")
            nc.vector.tensor_copy(m2_row, ps_m2)
            m2_ps = psLN2.tile([P, TQ], F32, tag="mbp2")
            nc.tensor.matmul(m2_ps, lhsT=onesr_sb, rhs=m2_row,
                             start=True, stop=True)
            ps_v2 = psLN2.tile([1, 512], F32, tag="st2")
            for kc in range(KC):
                nc.tensor.matmul(ps_v2, lhsT=cm_col, rhs=sq2[:, kc],
                                 start=(kc == 0), stop=(kc == KC - 1))
            msq2_row = ln2R.tile([1, TQ], F32, tag="row2")
            nc.scalar.square(msq2_row, m2_row.bitcast(F32))
            v2_row = ln2R.tile([1, TQ], F32, tag="row2")
            nc.vector.tensor_tensor(out=v2_row, in0=ps_v2, in1=msq2_row,
                                    op=ALU.subtract)
            zc = ln2T.tile([P, KC, TQ], BF16, tag="zc")
            nc.vector.tensor_tensor(
                out=zc, in0=y3_sb,
                in1=m2_ps[:, None, :].to_broadcast([P, KC, TQ]),
                op=ALU.subtract)
            ln2r = ln2R.tile([1, TQ], F32, tag="row2")
            nc.scalar.activation(ln2r, v2_row, AF.Ln, bias=eps1)
            rs2_row = ln2R.tile([1, TQ], F32R, tag="row2")
            nc.scalar.activation(rs2_row, ln2r, AF.Exp, scale=-0.5)
            rs2_ps = psLN2.tile([P, TQ], F32, tag="rsp2")
            nc.tensor.matmul(rs2_ps, lhsT=onesr_sb, rhs=rs2_row,
                             start=True, stop=True)
            nc.vector.tensor_tensor(
                out=z2_sb, in0=zc,
                in1=rs2_ps[:, None, :].to_broadcast([P, KC, TQ]),
                op=ALU.mult)

        # ======= Phase 7/8: MLP (bf16) ======================================
        with tc.tile_pool(name="gP", bufs=1) as gP, \
             tc.tile_pool(name="psMLP", bufs=3, space="PSUM") as psMLP, \
             tc.tile_pool(name="psOJ", bufs=1, space="PSUM") as psOJ:
            g_sb = gP.tile([P, KC4, TQ], BF16)
            out_sb = gP.tile([P, KC, TQ], F32)
            # fcproj accumulates per gelu chunk -> overlaps the fc phase
            oj = [psOJ.tile([P, 512], F32, tag=f"oj{j}", name=f"oj{j}")
                  for j in range(KC)]
            for mo in range(KC4):
                ps = psMLP.tile([P, 512], F32, tag="mm")
                for kc in range(KC):
                    nc.tensor.matmul(ps, lhsT=wfc_sb[:, kc, mo * P:(mo + 1) * P],
                                     rhs=z2_sb[:, kc],
                                     start=(kc == 0), stop=(kc == KC - 1))
                nc.scalar.activation(g_sb[:, mo], ps, AF.Gelu,
                                     bias=bfc_sb[:, mo:mo + 1])
                for j in range(KC):
                    nc.tensor.matmul(oj[j], lhsT=wfcp_sb[:, mo, j * P:(j + 1) * P],
                                     rhs=g_sb[:, mo],
                                     start=(mo == 0), stop=(mo == KC4 - 1))
            for j in range(KC):
                if j % 2 == 0:
                    nc.vector.tensor_scalar(out_sb[:, j], oj[j], bfcp_sb[:, j:j + 1],
                                            None, ALU.add)
                else:
                    nc.scalar.activation(out_sb[:, j], oj[j], AF.Identity,
                                         bias=bfcp_sb[:, j:j + 1])
                nc.sync.dma_start(outT_d[:, j, :], out_sb[:, j])

    nc.compile()
    return nc


def _fmt_bias(b):
    """[O] -> [128, O//128] per-partition layout."""
    return np.ascontiguousarray(b.reshape(-1, P).T)


def _prep_generic(inputs):
    f32 = np.float32
    x = np.asarray(inputs["x"], f32)
    coul = np.asarray(inputs["coulomb_matrix"], f32)
    g1 = np.asarray(inputs["ln1_g"], f32)
    b1 = np.asarray(inputs["ln1_b"], f32)
    g2 = np.asarray(inputs["ln2_g"], f32)
    b2 = np.asarray(inputs["ln2_b"], f32)
    wattn = np.asarray(inputs["w_attn"], f32)
    battn = np.asarray(inputs["b_attn"], f32)
    w_self = np.asarray(inputs["w_self"], f32)
    b_self = np.asarray(inputs["b_self"], f32)
    w_proj = np.asarray(inputs["w_proj"], f32)
    b_proj = np.asarray(inputs["b_proj"], f32)
    w_fc = np.asarray(inputs["w_fc"], f32)
    b_fc = np.asarray(inputs["b_fc"], f32)
    w_fcp = np.asarray(inputs["w_fc_proj"], f32)
    b_fcp = np.asarray(inputs["b_fc_proj"], f32)

    wq, wk, wv = wattn[:, 0:C], wattn[:, C:2 * C], wattn[:, 2 * C:]
    shared = {
        "wq": _fmt_lhs(g1[:, None] * wq).astype(ml_dtypes.bfloat16),
        "wk": _fmt_lhs(g1[:, None] * wk).astype(ml_dtypes.bfloat16),
        "wv": _fmt_lhs(g1[:, None] * wv).astype(ml_dtypes.bfloat16),
        "wself": _fmt_lhs(g1[:, None] * w_self).astype(ml_dtypes.bfloat16),
        "wproj": _fmt_lhs(w_proj).astype(ml_dtypes.bfloat16),
        "wfc": _fmt_lhs(g2[:, None] * w_fc).astype(ml_dtypes.bfloat16),
        "wfcp": _fmt_lhs(w_fcp).astype(ml_dtypes.bfloat16),
        "bq": _fmt_bias(battn[0:C] + b1 @ wq),
        "bk": _fmt_bias(battn[C:2 * C] + b1 @ wk),
        "bv": (battn[2 * C:] + b1 @ wv).reshape(1, C),
        "bself": _fmt_bias(b_self + b1 @ w_self),
        "bproj": _fmt_bias(b_proj),
        "bfc": _fmt_bias(b_fc + b2 @ w_fc),
        "bfcp": _fmt_bias(b_fcp),
        "cst": np.stack([np.ones(P, f32), np.full(P, 1.0 / C, f32)], axis=1).astype(ml_dtypes.bfloat16),
        "onesr": np.ones((1, P), f32),
    }
    in_maps = []
    for core in range(N_CORES):
        b, tqi = divmod(core, 4)
        tq0 = tqi * TQ
        xr = np.roll(x[b], -tq0, axis=0)                      # [T, C]
        xT = np.ascontiguousarray(
            xr.T.reshape(KC, P, T).transpose(1, 0, 2)).astype(
                ml_dtypes.bfloat16)                           # [P, KC, T]
        cr = np.roll(coul[b], -tq0, axis=1)[tq0:tq0 + TQ, :]  # [TQ, T]
        coulT = np.ascontiguousarray(
            cr.T.reshape(NTK, P, TQ)).astype(ml_dtypes.bfloat16)
        m = dict(shared)
        m["xT"] = xT
        m["coulT"] = coulT
        in_maps.append(m)
    return in_maps



